# revision 46
# baseline (speedup 1.0000x reference)
"""Bilateral filter v10 — full on-device stencil, PE-accumulated.

Computes the 7x7 bilateral directly from the padded image (no im2col, no
precomputed D/E maps: ~60MB -> ~10MB of DMA):
  - the 7 dy row-shifts are strided DMA replicas of one flat padded host
    array (engines cannot read at non-quadrant partition bases, DMA can
    gather anything from DRAM); two column-parity copies (A/B) keep every
    DVE read 4B-aligned for the bf16 2x mode
  - D = S - C on DVE (bf16 2x), w = DerivErf(sqrt50*D) on ACT fuses
    square+exp, T = w*D on DVE
  - +/-d pair symmetry halves the ACT work: w_{-d}(p) = w_d(p-d) and
    w_{-d}(p)*D_{-d}(p) = -T_d(p-d), so each of the 24 positive offsets
    serves both directions via shifted reads
  - accumulation over offsets runs on the Tensor engine: PSUM-accumulated
    matmuls whose stationaries are shifted diagonals +/-g_d*delta(k=j+3-dy)
    (spatial weight and the dy read-shift folded in for free; the dx
    read-shift is a free-axis offset on the moving operand); pure-vertical
    pairs (dy,0) read the same rhs for both directions and use merged
    2-banded stationaries; the two segments' tail 128-col chunks are
    accumulated by single merged N=256 matmuls
  - out = C + numD/(den + g0); PSUM drained on ACT (closest to PSUM, frees
    banks for the next half), reciprocal_approx_fast on DVE

Layout: padded image Ipad[486, 646] in segment tiles R[dy][par]: partition
p, segment k, col c -> Ipad[120k + p + dy, c + (0|1)], rows stored 656 wide
(4-col guard + 646 data + 6 guard) so every +/-3 col shift is in-bounds.
Pair order: (0,2) first (needs only the C replica), (3,0) last (cheapest
closing streams), den closes before num so the epilogue starts earlier.
"""
from contextlib import ExitStack
import os

import numpy as np
import ml_dtypes

import concourse.bass as bass
import concourse.bacc as bacc
import concourse.tile as tile
from concourse import mybir

F32 = mybir.dt.float32
BF16 = mybir.dt.bfloat16

H, W = 480, 640
PAD = 3
N_CORES = 8
SEG = 4              # row segments
RPS = 120            # output rows per segment
L = 656              # stored row length: 4 guard + 646 data + 6 guard
IH_ROWS = 497        # 1 guard row + 486 padded + 10 guard
SQRT50 = float(np.sqrt(50.0))

# positive half of the 7x7 offset set (24 offsets; negatives via symmetry).
# (0,2) first: its inputs are the first two DMA arrivals. (3,0) last: its
# merged standalone streams make the cheapest accumulation closing.
# QUADS: 4-member radius classes (equal g): their unshifted den streams are
# pre-summed on DVE and accumulated with ONE PE stream per group, using the
# first member's unshifted stationary (g*delta(j+3)) for the group.
QUADS = [
    [(2, -1), (2, 1), (1, -2), (1, 2)],
    [(3, -1), (3, 1), (1, -3), (1, 3)],
    [(3, -2), (3, 2), (2, -3), (2, 3)],
]
PAIRS = ([(0, 2), (0, 1), (0, 3)]
         + QUADS[0] + QUADS[1] + QUADS[2]
         + [(1, -1), (1, 1), (2, -2), (2, 2), (3, -3), (3, 3)]
         + [(1, 0), (2, 0), (3, 0)])
# (dy,-dx)/(dy,+dx) twins processed as one wide elementwise op; the merged
# tile's plane 0 holds -dx, plane 1 holds +dx
TWINS = [((dy, -dx), (dy, dx)) for dy in (1, 2, 3) for dx in (1, 2, 3)]
assert len(PAIRS) == 24 and len(set(PAIRS)) == 24
NQUAD = int(os.environ.get("K_NQUAD", "3"))
QUAD_POS = {d: (qi, ki) for qi, q in enumerate(QUADS[:NQUAD])
            for ki, d in enumerate(q)}

KNOB_BUFS = int(os.environ.get("K_BUFS", "4"))
KNOB_UF = int(os.environ.get("K_UF", "8"))   # hwloop body unroll


def make_IH(img):
    """[497, 656] bf16: guard row, then Ipad at col offset 4."""
    ih = np.zeros((IH_ROWS, L), np.float32)
    ih[1 + PAD:1 + PAD + H, 4 + PAD:4 + PAD + W] = np.asarray(img, np.float32)
    return ih.astype(ml_dtypes.bfloat16)


def make_G(g49):
    """[128, 72, 128] bf16 stationaries per pair i:
      3i+0: g_d * delta(k = j+3)       (unshifted streams)
      3i+1: g_d * delta(k = j+3-dy)    (shifted den stream)
      3i+2: -g_d * delta(k = j+3-dy)   (shifted num stream)
    dx==0 pairs use merged 2-banded den/num matrices in 3i+1 / 3i+2."""
    g49 = np.asarray(g49, np.float32).reshape(-1)
    G = np.zeros((128, 72, 128), np.float32)
    j = np.arange(120)
    for i, (dy, dx) in enumerate(PAIRS):
        g = float(g49[(dy + 3) * 7 + (dx + 3)])
        if dx == 0:
            G[j + 3, 3 * i + 1, j] += g
            G[j + 3 - dy, 3 * i + 1, j] += g
            G[j + 3, 3 * i + 2, j] += g
            G[j + 3 - dy, 3 * i + 2, j] += -g
        else:
            G[j + 3, 3 * i + 0, j] = g
            G[j + 3 - dy, 3 * i + 1, j] = g
            G[j + 3 - dy, 3 * i + 2, j] = -g
    return G.astype(ml_dtypes.bfloat16)


def emit(nc, IH_ap, G_ap, out_ap, g0=1.0, reps=1, hwloop=False):
    derf = mybir.ActivationFunctionType.Derivative_Erf

    def ih_src(dy, par):
        # R[dy][par][p, k, c] = IH[1 + 120k + p + dy, c - par]
        off = (1 + dy) * L - par
        return bass.AP(tensor=IH_ap.tensor, offset=IH_ap.offset + off,
                       ap=[[L, 128], [RPS * L, SEG], [1, L]])

    with tile.TileContext(nc) as tc, ExitStack() as ctx:
        singles = ctx.enter_context(tc.tile_pool(name="singles", bufs=1))
        gpool = ctx.enter_context(tc.tile_pool(name="gpool", bufs=2))
        dpool = ctx.enter_context(tc.tile_pool(name="dpool", bufs=KNOB_BUFS))
        wpool = ctx.enter_context(tc.tile_pool(name="wpool", bufs=KNOB_BUFS))
        tpool = ctx.enter_context(tc.tile_pool(name="tpool", bufs=KNOB_BUFS))
        ppool = ctx.enter_context(tc.tile_pool(name="ppool", bufs=1,
                                               space="PSUM"))
        opool = ctx.enter_context(tc.tile_pool(name="opool", bufs=2))
        apool = ctx.enter_context(tc.tile_pool(name="apool", bufs=2))
        d2pool = ctx.enter_context(tc.tile_pool(name="d2pool", bufs=3))
        w2pool = ctx.enter_context(tc.tile_pool(name="w2pool", bufs=3))
        t2pool = ctx.enter_context(tc.tile_pool(name="t2pool", bufs=3))

        # R[dy][0] = B copy (data at col 4), R[dy][1] = A copy (col 5)
        R = [[singles.tile([128, SEG, L], BF16, name=f"R{dy}{par}")
              for par in range(2)] for dy in range(4)]
        G_t = gpool.tile([128, 72, 128], BF16, name="G_t")

        warm = singles.tile([128, 2], BF16, name="warm")

        def body():
            # warm the Derivative_Erf ACT spline table during the input DMAs
            nc.scalar.activation(out=warm, in_=warm, func=derf, bias=0.0,
                                 scale=SQRT50)
            # replicas (first-use order) on SP queue; G chunks on gpsimd
            for dy, par in ((0, 0), (0, 1), (1, 0), (2, 1), (1, 1), (3, 1),
                            (2, 0), (3, 0)):
                nc.sync.dma_start(out=R[dy][par], in_=ih_src(dy, par))
            for i in range(24):
                nc.gpsimd.dma_start(out=G_t[:, 3 * i:3 * i + 3, :],
                                    in_=G_ap[:, 3 * i:3 * i + 3, :])
            C_B = R[0][0]
            for h in range(2):
                n512 = ppool.tile([128, 2, 512], F32, name="n512")
                n128 = ppool.tile([128, 2, 128], F32, name="n128")
                d512 = ppool.tile([128, 2, 512], F32, name="d512")
                d128 = ppool.tile([128, 2, 128], F32, name="d128")
                w_first = [None] * len(QUADS)
                wacc = [None] * len(QUADS)
                def mk_streams(a512_a128_pairs):
                    pass

                def emit_streams(a512, a128, stat, src_t, pl, coff, st, sp):
                    # pl None: src_t is [128,2,L]; else plane pl of [128,2,2,L]
                    for m in range(2):
                        rhs = (src_t[0:123, m, coff:coff + 512] if pl is None
                               else src_t[0:123, pl, m, coff:coff + 512])
                        nc.tensor.matmul(a512[:, m, :], stat, rhs,
                                         start=st, stop=sp)
                    rhs = (src_t[0:123, :, coff + 512:coff + 640] if pl is None
                           else src_t[0:123, pl, :, coff + 512:coff + 640])
                    nc.tensor.matmul(a128[:, :, :], stat, rhs,
                                     start=st, stop=sp)

                def pair_streams(i, dy, dx, W, T, pl, first, last):
                    gp0 = G_t[0:123, 3 * i + 0, :]
                    gps = G_t[0:123, 3 * i + 1, :]
                    gns = G_t[0:123, 3 * i + 2, :]
                    if dx == 0:
                        emit_streams(d512, d128, gps, W, pl, 7, False, last)
                        emit_streams(n512, n128, gns, T, pl, 7, False, last)
                        return
                    if (dy, dx) not in QUAD_POS:
                        emit_streams(d512, d128, gp0, W, pl, 7, first, False)
                    emit_streams(n512, n128, gp0, T, pl, 7, first, False)
                    emit_streams(d512, d128, gps, W, pl, 7 - dx, False, False)
                    emit_streams(n512, n128, gns, T, pl, 7 - dx, False, False)

                qpart = [None] * len(QUADS)
                i = 0
                while i < len(PAIRS):
                    dy, dx = PAIRS[i]
                    twin = (i + 1 < len(PAIRS)
                            and PAIRS[i + 1] == (dy, -dx) and dx < 0)
                    par = 1 if dx % 2 else 0
                    if twin:
                        # planes: 0 = (dy,dx<0), 1 = (dy,-dx)
                        c0m = 4 + par + dx          # dx negative
                        base = R[dy][par][:]
                        s_ap = bass.AP(
                            tensor=base.tensor,
                            offset=base.offset + 2 * h * L + c0m,
                            ap=[[base.ap[0][0], 123], [-2 * dx, 2], [L, 2],
                                [1, 646]])
                        cb = C_B[:]
                        c_ap = bass.AP(
                            tensor=cb.tensor,
                            offset=cb.offset + 2 * h * L + 4,
                            ap=[[cb.ap[0][0], 123], [0, 2], [L, 2],
                                [1, 646]])
                        D_t = d2pool.tile([128, 2, 2, L], BF16, name="D2")
                        nc.vector.tensor_tensor(
                            out=D_t[0:123, :, :, 4:650], in0=s_ap, in1=c_ap,
                            op=mybir.AluOpType.subtract)
                        W_t = w2pool.tile([128, 2, 2, L], BF16, name="W2")
                        nc.scalar.activation(
                            out=W_t[0:123, :, :, 4:650],
                            in_=D_t[0:123, :, :, 4:650],
                            func=derf, bias=0.0, scale=SQRT50)
                        T_t = t2pool.tile([128, 2, 2, L], BF16, name="T2")
                        nc.vector.tensor_tensor(
                            out=T_t[0:123, :, :, 4:650],
                            in0=W_t[0:123, :, :, 4:650],
                            in1=D_t[0:123, :, :, 4:650],
                            op=mybir.AluOpType.mult)
                        pair_streams(i, dy, dx, W_t, T_t, 0, False, False)
                        pair_streams(i + 1, dy, -dx, W_t, T_t, 1,
                                     False, False)
                        if (dy, dx) in QUAD_POS:
                            qi, ki = QUAD_POS[(dy, dx)]
                            ts = apool.tile([128, 2, L], BF16, name="ts")
                            nc.vector.tensor_tensor(
                                out=ts[0:123, :, 4:650],
                                in0=W_t[0:123, 0, :, 4:650],
                                in1=W_t[0:123, 1, :, 4:650],
                                op=mybir.AluOpType.add)
                            if qpart[qi] is None:
                                qpart[qi] = ts
                            else:
                                wacc = apool.tile([128, 2, L], BF16,
                                                  name="wacc")
                                nc.vector.tensor_tensor(
                                    out=wacc[0:123, :, 4:650],
                                    in0=qpart[qi][0:123, :, 4:650],
                                    in1=ts[0:123, :, 4:650],
                                    op=mybir.AluOpType.add)
                                gslot = 3 * PAIRS.index(QUADS[qi][0])
                                emit_streams(d512, d128,
                                             G_t[0:123, gslot, :],
                                             wacc, None, 7, False, False)
                                qpart[qi] = None
                        i += 2
                        continue
                    c0 = 4 + par + dx
                    D_t = dpool.tile([128, 2, L], BF16, name="D")
                    nc.vector.tensor_tensor(
                        out=D_t[0:123, :, 4:650],
                        in0=R[dy][par][0:123, 2 * h:2 * h + 2, c0:c0 + 646],
                        in1=C_B[0:123, 2 * h:2 * h + 2, 4:650],
                        op=mybir.AluOpType.subtract)
                    W_t = wpool.tile([128, 2, L], BF16, name="Wt")
                    nc.scalar.activation(
                        out=W_t[0:123, :, 4:650], in_=D_t[0:123, :, 4:650],
                        func=derf, bias=0.0, scale=SQRT50)
                    T_t = tpool.tile([128, 2, L], BF16, name="Tt")
                    nc.vector.tensor_tensor(
                        out=T_t[0:123, :, 4:650], in0=W_t[0:123, :, 4:650],
                        in1=D_t[0:123, :, 4:650], op=mybir.AluOpType.mult)
                    pair_streams(i, dy, dx, W_t, T_t, None,
                                 i == 0, i == 23)
                    i += 1
                # drain PSUM on ACT, then finish the math on DVE
                # drain both segments of den first: the d-tiles free as a
                # whole, and the next half's first matmuls are den streams
                denf, numf = [], []
                for m in range(2):
                    df = opool.tile([120, 640], F32, name=f"denf{m}")
                    nc.scalar.activation(
                        out=df[:, 0:512], in_=d512[0:120, m, :],
                        func=mybir.ActivationFunctionType.Copy, bias=g0)
                    nc.scalar.activation(
                        out=df[:, 512:640], in_=d128[0:120, m, :],
                        func=mybir.ActivationFunctionType.Copy, bias=g0)
                    denf.append(df)
                for m in range(2):
                    nf = opool.tile([120, 640], F32, name=f"numf{m}")
                    nc.scalar.copy(out=nf[:, 0:512], in_=n512[0:120, m, :])
                    nc.scalar.copy(out=nf[:, 512:640], in_=n128[0:120, m, :])
                    numf.append(nf)
                for m in range(2):
                    rt = 2 * h + m
                    rec = opool.tile([120, 640], F32, name="rec")
                    nc.vector.reciprocal_approx_fast(out=rec, in_=denf[m])
                    q = opool.tile([120, 640], F32, name="q")
                    qeng = nc.gpsimd if int(os.environ.get("K_EPI_POOL", "0")) \
                        else nc.vector
                    qeng.tensor_tensor(out=q, in0=numf[m], in1=rec,
                                       op=mybir.AluOpType.mult)
                    o_t = opool.tile([120, 640], F32, name="o")
                    # C on the output rows: Ipad[120rt + 3 + j, 3 + x]
                    qeng.tensor_tensor(
                        out=o_t, in0=q, in1=R[3][0][0:120, rt, 7:647],
                        op=mybir.AluOpType.add)
                    nc.sync.dma_start(
                        out=out_ap[120 * rt:120 * rt + 120, :], in_=o_t)

        if hwloop and reps > 1:
            uf = KNOB_UF
            if reps % uf:
                uf = 1
            with tc.For_i(0, reps // uf):
                for _ in range(uf):
                    body()
        else:
            for _ in range(reps):
                body()


def build_nc(reps=1, hwloop=False, g0=1.0):
    nc = bacc.Bacc(num_devices=N_CORES)
    IH = nc.dram_tensor("IH", [IH_ROWS, L], BF16, kind="ExternalInput")
    G = nc.dram_tensor("G", [128, 72, 128], BF16, kind="ExternalInput")
    out = nc.dram_tensor("out", [H, W], F32, kind="ExternalOutput")
    emit(nc, IH.ap(), G.ap(), out.ap(), g0=g0, reps=reps, hwloop=hwloop)
    nc.finalize()
    return nc


def make_in_maps(I, g49):
    G = make_G(g49)
    return [{"IH": make_IH(I[c, 0]), "G": G} for c in range(I.shape[0])]


def kernel(I: np.ndarray, g: np.ndarray) -> np.ndarray:
    from concourse.bass_utils import run_bass_kernel_spmd

    I = np.ascontiguousarray(np.asarray(I, np.float32))
    g49 = np.asarray(g, np.float32).reshape(-1)
    nc = build_nc(g0=float(g49[24]))
    in_maps = make_in_maps(I, g49)
    res = run_bass_kernel_spmd(nc, in_maps, core_ids=list(range(N_CORES)))
    return np.stack([r["out"] for r in res.results], axis=0)
